# revision 1
# baseline (speedup 1.0000x reference)
"""CrossModalityAttention Trainium2 Bass kernel.

Data-parallel over batch: 8 cores, one batch element each.
Per core (b): out[b] = softmax((img[b]@Wq + bq) @ (txt[b]@Wk + bk)^T / 32) @ (txt[b]@Wv + bv)

Layout strategy (everything oriented so matmul contraction = partition dim):
  txtT[d, k], imgT[d, q] : PE-transposed inputs (fp32 transpose via identity)
  V0[k, h] = txt Wv      : lhsT = txtT slice,   rhs = Wv (natural) -> SBUF resident
                           (bias bv folded into the epilogue: out = O0/sums + bv)
  Kt[h, k] = Wk^T txt^T  : lhsT = Wk (natural), rhs = txtT         -> SBUF resident
  Qt[h, q] = Wq^T img^T  : lhsT = Wq (natural), rhs = imgT         -> DRAM scratch
  St[k, q] = Kt^T Qt     : lhsT = Kt slice,     rhs = Qt chunk
  E = exp(St / 32)       : ACT, PSUM -> SBUF (fp32r)
  O0[q, h] = E^T V0      : lhsT = E slice,      rhs = V0 tile
  sums[q]  = E^T ones    : lhsT = E slice,      rhs = ones
  out rows = O0 / sums + bv : DVE, PSUM -> SBUF -> DRAM

All matmuls use float32r (TF32) operands, fp32 PSUM accumulation.
Phase order T(txtT) -> V -> Kt -> Q -> A keeps PE dense: the only DMA on the
critical path at startup is the first 384KB of txt rows.
"""

import numpy as np

import concourse.bass as bass
import concourse.tile as tile
from concourse import bacc, mybir
from concourse.bass_utils import run_bass_kernel_spmd
from concourse.masks import make_identity

F32 = mybir.dt.float32
F32R = mybir.dt.float32r
AF = mybir.ActivationFunctionType

P = 128
B, LQ, LK = 8, 2048, 2048
IMG, TXT, HID = 1024, 768, 1024
NQT, NKT = LQ // P, LK // P            # 16, 16 seq tiles
NIC, NTC = IMG // P, TXT // P          # 8, 6 contraction chunks
NHT = HID // P                         # 8 hid tiles
QC = 512                               # q chunk width
NQC = LQ // QC                         # 4
SCALE = 1.0 / np.sqrt(np.float32(HID))

_CACHED = {}


def build_kernel(reps=1):
    nc = bacc.Bacc("TRN2", target_bir_lowering=False, debug=False)
    img = nc.dram_tensor("img", [LQ, IMG], F32, kind="ExternalInput").ap()
    txt = nc.dram_tensor("txt", [LK, TXT], F32, kind="ExternalInput").ap()
    wq = nc.dram_tensor("wq", [IMG, HID], F32, kind="ExternalInput").ap()
    wk = nc.dram_tensor("wk", [TXT, HID], F32, kind="ExternalInput").ap()
    wv = nc.dram_tensor("wv", [TXT, HID], F32, kind="ExternalInput").ap()
    bq = nc.dram_tensor("bq", [HID], F32, kind="ExternalInput").ap()
    bk = nc.dram_tensor("bk", [HID], F32, kind="ExternalInput").ap()
    bv = nc.dram_tensor("bv", [HID], F32, kind="ExternalInput").ap()
    out = nc.dram_tensor("out_attn", [LQ, HID], F32, kind="ExternalOutput").ap()

    with tile.TileContext(nc) as tc:
        with (
            tc.tile_pool(name="persist", bufs=1) as persist,
            tc.tile_pool(name="dram", bufs=1, space="DRAM") as dram,
            tc.tile_pool(name="psum", bufs=3, space="PSUM") as psum,
        ):
            ident = persist.tile([P, P], F32, tag="ident")
            make_identity(nc, ident[:])
            ones_f = persist.tile([P, 2], F32, tag="ones_f")
            nc.vector.memset(ones_f[:], 1.0)
            ones = persist.tile([P, 2], F32R, tag="ones")
            nc.vector.tensor_copy(ones[:], ones_f[:])
            bq_t = persist.tile([P, NHT], F32, tag="bq")
            bk_t = persist.tile([P, NHT], F32, tag="bk")
            nc.gpsimd.dma_start(out=bq_t[:], in_=bq.rearrange("(t p) -> p t", p=P))
            nc.gpsimd.dma_start(out=bk_t[:], in_=bk.rearrange("(t p) -> p t", p=P))
            bv_bc = persist.tile([P, HID], F32, tag="bv")
            nc.gpsimd.dma_start(out=bv_bc[:], in_=bv.partition_broadcast(P))
            kt_t = [persist.tile([P, LK], F32R, tag=f"kt{h}", name=f"kt{h}")
                    for h in range(NHT)]

            qt_d = dram.tile([HID, LQ], F32R)
            v_d = dram.tile([LK, HID], F32R)

            for rep in range(reps):
                # ------------- Phase T: txtT (PE transpose of txt) ----------------
                with (
                    tc.tile_pool(name=f"tkv{rep}", bufs=1) as tkv,
                    tc.tile_pool(name=f"tst{rep}", bufs=2) as tst,
                ):
                    txtT = [tkv.tile([P, LK], F32R, tag=f"txtT{c}", name=f"txtT{c}")
                            for c in range(NTC)]
                    for g in range(4):
                        rt = []
                        for r in range(4):
                            t = tst.tile([P, TXT], F32, tag=f"txtrow{r}", name=f"txtrow{r}")
                            nc.sync.dma_start(
                                out=t[:], in_=txt[(g * 4 + r) * P:(g * 4 + r + 1) * P, :]
                            )
                            rt.append(t)
                        for c in range(NTC):
                            ps = psum.tile([P, QC], F32, tag="pt", name="pt")
                            for r in range(4):
                                nc.tensor.transpose(
                                    ps[:, r * P:(r + 1) * P],
                                    rt[r][:, c * P:(c + 1) * P],
                                    ident[:],
                                )
                            nc.vector.tensor_copy(txtT[c][:, g * QC:(g + 1) * QC], ps[:])

                    # ------------- Phase V: V0 = txt @ Wv (no bias) -> SBUF -------
                    wv_r = []
                    for c in range(NTC):
                        t = tst.tile([P, HID], F32, tag="wst", bufs=2, name="wvst")
                        nc.sync.dma_start(out=t[:], in_=wv[c * P:(c + 1) * P, :])
                        w = tkv.tile([P, HID], F32R, tag=f"wv{c}", name=f"wvr{c}")
                        nc.vector.tensor_copy(w[:], t[:])
                        wv_r.append(w)
                    for k in range(NKT):
                        vs = tst.tile([P, HID], F32R, tag="vst", bufs=2, name="vst")
                        for hc in range(HID // QC):
                            ps = psum.tile([P, QC], F32, tag="pt", name="pt")
                            for c in range(NTC):
                                nc.tensor.matmul(
                                    ps[:],
                                    txtT[c][:, k * P:(k + 1) * P],
                                    wv_r[c][:, hc * QC:(hc + 1) * QC],
                                    start=(c == 0),
                                    stop=(c == NTC - 1),
                                )
                            if (k + hc) % 2:
                                nc.vector.tensor_copy(
                                    vs[:, hc * QC:(hc + 1) * QC], ps[:]
                                )
                            else:
                                nc.scalar.copy(vs[:, hc * QC:(hc + 1) * QC], ps[:])
                        nc.sync.dma_start(out=v_d[k * P:(k + 1) * P, :], in_=vs[:])

                    # ------------- Phase K: Kt = Wk^T @ txtT + bk -> SBUF ---------
                    wk_r = []
                    for c in range(NTC):
                        t = tst.tile([P, HID], F32, tag="wst", bufs=2, name="wkst")
                        nc.sync.dma_start(out=t[:], in_=wk[c * P:(c + 1) * P, :])
                        w = tkv.tile([P, HID], F32R, tag=f"wk{c}", name=f"wkr{c}")
                        nc.vector.tensor_copy(w[:], t[:])
                        wk_r.append(w)
                    for h in range(NHT):
                        for kc in range(LK // QC):
                            ps = psum.tile([P, QC], F32, tag="pt", name="pt")
                            for c in range(NTC):
                                nc.tensor.matmul(
                                    ps[:],
                                    wk_r[c][:, h * P:(h + 1) * P],
                                    txtT[c][:, kc * QC:(kc + 1) * QC],
                                    start=(c == 0),
                                    stop=(c == NTC - 1),
                                )
                            nc.vector.tensor_scalar_add(
                                kt_t[h][:, kc * QC:(kc + 1) * QC], ps[:], bk_t[:, h:h + 1]
                            )

                # ------------- V readback (overlaps phase Q) ----------------------
                with tc.tile_pool(name=f"vload{rep}", bufs=1) as vload:
                    # --------- Phase Q: Qt = Wq^T @ imgT + bq -> DRAM -------------
                    with (
                        tc.tile_pool(name=f"wqp{rep}", bufs=1) as wqp,
                        tc.tile_pool(name=f"qst{rep}", bufs=2) as qst,
                    ):
                        wq_r = []
                        for c in range(NIC):
                            t = qst.tile([P, HID], F32, tag="wst", bufs=1, name="wqst")
                            nc.sync.dma_start(out=t[:], in_=wq[c * P:(c + 1) * P, :])
                            w = wqp.tile([P, HID], F32R, tag=f"wq{c}", name=f"wqr{c}")
                            nc.vector.tensor_copy(w[:], t[:])
                            wq_r.append(w)

                        v_t = [None] * NKT

                        for qc in range(NQC):
                            rt = []
                            for r in range(4):
                                t = qst.tile([P, IMG], F32, tag=f"imgrow{r}", bufs=1, name=f"imgrow{r}")
                                nc.sync.dma_start(
                                    out=t[:], in_=img[qc * QC + r * P: qc * QC + (r + 1) * P, :]
                                )
                                rt.append(t)
                            imgT = []
                            for c in range(NIC):
                                ps = psum.tile([P, QC], F32, tag="pt", name="pt")
                                for r in range(4):
                                    nc.tensor.transpose(
                                        ps[:, r * P:(r + 1) * P],
                                        rt[r][:, c * P:(c + 1) * P],
                                        ident[:],
                                    )
                                it = qst.tile([P, QC], F32R, tag=f"imgT{c}", bufs=1, name=f"imgT{c}")
                                nc.vector.tensor_copy(it[:], ps[:])
                                imgT.append(it)
                            for k in range(qc * 4, qc * 4 + 4):
                                t = vload.tile([P, HID], F32R, tag=f"v{k}", name=f"vt{k}")
                                nc.sync.dma_start(out=t[:], in_=v_d[k * P:(k + 1) * P, :])
                                v_t[k] = t
                            for h in range(NHT):
                                ps = psum.tile([P, QC], F32, tag="pt", name="pt")
                                for c in range(NIC):
                                    nc.tensor.matmul(
                                        ps[:],
                                        wq_r[c][:, h * P:(h + 1) * P],
                                        imgT[c][:],
                                        start=(c == 0),
                                        stop=(c == NIC - 1),
                                    )
                                qs = qst.tile([P, QC], F32R, tag="qtst", bufs=3, name="qtst")
                                nc.vector.tensor_scalar_add(qs[:], ps[:], bq_t[:, h:h + 1])
                                nc.sync.dma_start(
                                    out=qt_d[h * P:(h + 1) * P, qc * QC:(qc + 1) * QC],
                                    in_=qs[:],
                                )

                    # ------------- Phase A: attention ---------------------------------
                    with tc.tile_pool(name=f"attn{rep}", bufs=1) as attn:
                        for qc in range(NQC):
                            qt_c = []
                            for h in range(NHT):
                                t = attn.tile([P, QC], F32R, tag=f"qt{h}", bufs=2,
                                              name=f"qtc{h}")
                                nc.sync.dma_start(
                                    out=t[:],
                                    in_=qt_d[h * P:(h + 1) * P, qc * QC:(qc + 1) * QC],
                                )
                                qt_c.append(t)
                            e_t = []
                            for k in range(NKT):
                                ps = psum.tile([P, QC], F32, tag="pt", name="pt")
                                for h in range(NHT):
                                    nc.tensor.matmul(
                                        ps[:],
                                        kt_t[h][:, k * P:(k + 1) * P],
                                        qt_c[h][:],
                                        start=(h == 0),
                                        stop=(h == NHT - 1),
                                    )
                                e = attn.tile([P, QC], F32R, tag=f"e{k}", name=f"e{k}")
                                nc.scalar.activation(e[:], ps[:], AF.Exp, scale=float(SCALE))
                                e_t.append(e)
                            for qs in range(QC // P):
                                po = psum.tile([P, HID], F32, tag="po", bufs=2, name="po")
                                pn = psum.tile([P, 2], F32, tag="pn", bufs=1, name="pn")
                                for k in range(NKT):
                                    esl = e_t[k][:, qs * P:(qs + 1) * P]
                                    nc.tensor.matmul(
                                        po[:, 0:QC], esl, v_t[k][:, 0:QC],
                                        start=(k == 0), stop=(k == NKT - 1),
                                    )
                                    nc.tensor.matmul(
                                        po[:, QC:HID], esl, v_t[k][:, QC:HID],
                                        start=(k == 0), stop=(k == NKT - 1),
                                    )
                                    nc.tensor.matmul(
                                        pn[:], esl, ones[:],
                                        start=(k == 0), stop=(k == NKT - 1),
                                    )
                                rs = attn.tile([P, 1], F32, tag="rs", bufs=2, name="rs")
                                nc.vector.reciprocal(rs[:], pn[:, 0:1])
                                ot = attn.tile([P, HID], F32, tag="ot", bufs=2, name="ot")
                                nc.vector.tensor_scalar_mul(ot[:], po[:], rs[:])
                                nc.vector.tensor_add(ot[:], ot[:], bv_bc[:])
                                row = qc * QC + qs * P
                                nc.sync.dma_start(out=out[row:row + P, :], in_=ot[:])


    nc.compile()
    return nc


def _get_nc():
    if "nc" not in _CACHED:
        _CACHED["nc"] = build_kernel()
    return _CACHED["nc"]


def kernel(image_features, text_features, Wq, bq, Wk, bk, Wv, bv):
    img = np.ascontiguousarray(np.asarray(image_features, np.float32))
    txt = np.ascontiguousarray(np.asarray(text_features, np.float32))
    shared = {
        "wq": np.ascontiguousarray(np.asarray(Wq, np.float32)),
        "wk": np.ascontiguousarray(np.asarray(Wk, np.float32)),
        "wv": np.ascontiguousarray(np.asarray(Wv, np.float32)),
        "bq": np.ascontiguousarray(np.asarray(bq, np.float32)),
        "bk": np.ascontiguousarray(np.asarray(bk, np.float32)),
        "bv": np.ascontiguousarray(np.asarray(bv, np.float32)),
    }
    in_maps = [{"img": img[b], "txt": txt[b], **shared} for b in range(B)]
    res = run_bass_kernel_spmd(_get_nc(), in_maps, core_ids=list(range(B)))
    return np.stack([res.results[b]["out_attn"] for b in range(B)])



# revision 3
# speedup vs baseline: 1.2964x; 1.2964x over previous
"""CrossModalityAttention Trainium2 Bass kernel.

Data-parallel over batch: 8 cores, one batch element each.
Per core (b): out[b] = softmax((img[b]@Wq + bq) @ (txt[b]@Wk + bk)^T / 32) @ (txt[b]@Wv + bv)

Key choices vs the fp32r baseline (545us):
  * All matmul operands bf16 (host casts img/txt/W* to bf16). fp32 PSUM
    accumulation. Measured end-to-end rel err ~3e-3 << 2e-2 budget.
  * bk dropped entirely: S[q,k] = Q.K[k] + Q.bk is a per-row constant shift
    under row-softmax, so it cancels.
  * imgT / txtT produced by XBAR DMA transpose (16x128-tile crossbar, 2-byte
    dtypes) straight from DRAM -- zero PE transpose work, no identity matrix.
  * Everything SBUF-resident: txtT 3MB, V 4MB, Kt 4MB, weights 5MB, per-qc
    Qt/imgT/E double-buffered. No DRAM scratch round-trips.
  * Phase order V -> Kt -> per-q-chunk [Qt -> S -> exp -> O -> epilogue],
    emitted so the PE never waits on anything but the first ~2MB of DMA.

Layout (contraction dim always = partition dim):
  txtT[d, k] (XBAR)      imgT[i, q-chunk] (XBAR, double-buffered)
  V[k, h]    = txtT^T Wv               -> SBUF bf16 (bv folded in epilogue)
  Kt[h, k]   = Wk^T txtT               -> SBUF bf16 (no bk)
  Qt[h, q]   = Wq^T imgT + bq          -> SBUF bf16, per q-chunk
  S[k, q]    = Kt^T Qt   (psum f32)
  E = exp(S/32)          (ACT, psum -> SBUF bf16)
  O[q, h]    = E^T V, sums[q] = E^T ones, out = O/sums + bv
"""

import numpy as np
import ml_dtypes

import concourse.bass as bass
import concourse.tile as tile
from concourse import bacc, mybir
from concourse.bass_utils import run_bass_kernel_spmd

F32 = mybir.dt.float32
BF16 = mybir.dt.bfloat16
AF = mybir.ActivationFunctionType

P = 128
B, LQ, LK = 8, 2048, 2048
IMG, TXT, HID = 1024, 768, 1024
NKT = LK // P                 # 16 key tiles
NTC = TXT // P                # 6 txt contraction chunks
NIC = IMG // P                # 8 img contraction chunks
NHT = HID // P                # 8 hid tiles
QC = 512                      # q chunk width
NQC = LQ // QC                # 4
SCALE = 1.0 / np.sqrt(np.float32(HID))

_CACHED = {}


def build_kernel(reps=1):
    nc = bacc.Bacc("TRN2", target_bir_lowering=False, debug=False)
    img = nc.dram_tensor("img", [LQ, IMG], BF16, kind="ExternalInput").ap()
    txt = nc.dram_tensor("txt", [LK, TXT], BF16, kind="ExternalInput").ap()
    wq = nc.dram_tensor("wq", [IMG, HID], BF16, kind="ExternalInput").ap()
    wk = nc.dram_tensor("wk", [TXT, HID], BF16, kind="ExternalInput").ap()
    wv = nc.dram_tensor("wv", [TXT, HID], BF16, kind="ExternalInput").ap()
    bq = nc.dram_tensor("bq", [HID], F32, kind="ExternalInput").ap()
    bv = nc.dram_tensor("bv", [HID], F32, kind="ExternalInput").ap()
    out = nc.dram_tensor("out_attn", [LQ, HID], F32, kind="ExternalOutput").ap()

    with tile.TileContext(nc) as tc:
        with (
            tc.tile_pool(name="sb", bufs=1) as sb,
            tc.tile_pool(name="psum", bufs=1, space="PSUM") as psum,
        ):
            ones = sb.tile([P, 2], BF16, tag="ones")
            nc.vector.memset(ones[:], 1.0)
            bq_t = sb.tile([P, NHT], F32, tag="bq")
            nc.gpsimd.dma_start(out=bq_t[:], in_=bq.rearrange("(t p) -> p t", p=P))
            bv_bc = sb.tile([P, HID], F32, tag="bv")
            nc.gpsimd.dma_start(out=bv_bc[:], in_=bv.partition_broadcast(P))

            v_t = [sb.tile([P, HID], BF16, tag=f"v{k}", name=f"v{k}")
                   for k in range(NKT)]
            kt_t = [sb.tile([P, LK], BF16, tag=f"kt{h}", name=f"kt{h}")
                    for h in range(NHT)]

            for rep in range(reps):
                with tc.tile_pool(name=f"proj{rep}", bufs=1) as proj:
                    # ---- weight + txtT loads (order = DMA priority) -------
                    wv_t = []
                    for c in range(NTC):
                        t = proj.tile([P, HID], BF16, tag=f"wv{c}", name=f"wv{c}")
                        nc.sync.dma_start(out=t[:], in_=wv[c * P:(c + 1) * P, :])
                        wv_t.append(t)
                    txtT = [proj.tile([P, LK], BF16, tag=f"txtT{c}", name=f"txtT{c}")
                            for c in range(NTC)]
                    # first k-quarter of txtT right behind wv
                    for g in range(4):
                        for c in range(NTC):
                            nc.sync.dma_start(
                                out=txtT[c][:, g * QC:(g + 1) * QC],
                                in_=txt[g * QC:(g + 1) * QC, c * P:(c + 1) * P],
                                transpose=True,
                            )
                        if g == 0:
                            wk_t = []
                            for c in range(NTC):
                                t = proj.tile([P, HID], BF16, tag=f"wk{c}",
                                              name=f"wk{c}")
                                nc.sync.dma_start(out=t[:],
                                                  in_=wk[c * P:(c + 1) * P, :])
                                wk_t.append(t)
                    wq_t = []
                    for c in range(NIC):
                        t = sb.tile([P, HID], BF16, tag=f"wq{c}", name=f"wq{c}")
                        nc.sync.dma_start(out=t[:], in_=wq[c * P:(c + 1) * P, :])
                        wq_t.append(t)

                    # ---- Phase V: V[k,h] = txtT^T Wv ----------------------
                    for k in range(NKT):
                        for hc in range(HID // QC):
                            ps = psum.tile([P, QC], F32, tag="pt", bufs=2,
                                           name="pt")
                            for c in range(NTC):
                                nc.tensor.matmul(
                                    ps[:],
                                    txtT[c][:, k * P:(k + 1) * P],
                                    wv_t[c][:, hc * QC:(hc + 1) * QC],
                                    start=(c == 0),
                                    stop=(c == NTC - 1),
                                )
                            dst = v_t[k][:, hc * QC:(hc + 1) * QC]
                            if (k + hc) % 2:
                                nc.vector.tensor_copy(dst, ps[:])
                            else:
                                nc.scalar.copy(dst, ps[:])

                    # ---- Phase K: Kt[h,k] = Wk^T txtT (no bk) -------------
                    for h in range(NHT):
                        for kc in range(LK // QC):
                            ps = psum.tile([P, QC], F32, tag="pt", bufs=2,
                                           name="pt")
                            for c in range(NTC):
                                nc.tensor.matmul(
                                    ps[:],
                                    wk_t[c][:, h * P:(h + 1) * P],
                                    txtT[c][:, kc * QC:(kc + 1) * QC],
                                    start=(c == 0),
                                    stop=(c == NTC - 1),
                                )
                            dst = kt_t[h][:, kc * QC:(kc + 1) * QC]
                            if (h + kc) % 2:
                                nc.vector.tensor_copy(dst, ps[:])
                            else:
                                nc.scalar.copy(dst, ps[:])

                # ---- per-q-chunk: Qt -> S -> exp -> O -> epilogue ---------
                with tc.tile_pool(name=f"attn{rep}", bufs=1) as attn:
                    def load_imgT(qc):
                        tiles = []
                        for c in range(NIC):
                            t = attn.tile([P, QC], BF16, tag=f"imgT{c}", bufs=2,
                                          name=f"imgT{c}")
                            nc.sync.dma_start(
                                out=t[:],
                                in_=img[qc * QC:(qc + 1) * QC,
                                        c * P:(c + 1) * P],
                                transpose=True,
                            )
                            tiles.append(t)
                        return tiles

                    imgT = load_imgT(0)
                    for qc in range(NQC):
                        # Qt for this chunk
                        qt = []
                        for h in range(NHT):
                            ps = psum.tile([P, QC], F32, tag="pt", bufs=2,
                                           name="pt")
                            for c in range(NIC):
                                nc.tensor.matmul(
                                    ps[:],
                                    wq_t[c][:, h * P:(h + 1) * P],
                                    imgT[c][:],
                                    start=(c == 0),
                                    stop=(c == NIC - 1),
                                )
                            qh = attn.tile([P, QC], BF16, tag=f"qt{h}", bufs=2,
                                           name=f"qt{h}")
                            nc.vector.tensor_scalar_add(qh[:], ps[:],
                                                        bq_t[:, h:h + 1])
                            qt.append(qh)
                        # prefetch imgT for next chunk
                        if qc + 1 < NQC:
                            imgT = load_imgT(qc + 1)
                        # S + exp
                        e_t = []
                        for k in range(NKT):
                            ps = psum.tile([P, QC], F32, tag="pt", bufs=2,
                                           name="pt")
                            for h in range(NHT):
                                nc.tensor.matmul(
                                    ps[:],
                                    kt_t[h][:, k * P:(k + 1) * P],
                                    qt[h][:],
                                    start=(h == 0),
                                    stop=(h == NHT - 1),
                                )
                            e = attn.tile([P, QC], BF16, tag=f"e{k}", bufs=2,
                                          name=f"e{k}")
                            nc.scalar.activation(e[:], ps[:], AF.Exp,
                                                 scale=float(SCALE))
                            e_t.append(e)
                        # O + row sums + epilogue
                        for qs in range(QC // P):
                            po0 = psum.tile([P, QC], F32, tag="po0", bufs=2,
                                            name="po0")
                            po1 = psum.tile([P, QC], F32, tag="po1", bufs=2,
                                            name="po1")
                            pn = psum.tile([P, 2], F32, tag="pn", bufs=2,
                                           name="pn")
                            for k in range(NKT):
                                esl = e_t[k][:, qs * P:(qs + 1) * P]
                                nc.tensor.matmul(
                                    po0[:], esl, v_t[k][:, 0:QC],
                                    start=(k == 0), stop=(k == NKT - 1),
                                )
                                nc.tensor.matmul(
                                    po1[:], esl, v_t[k][:, QC:HID],
                                    start=(k == 0), stop=(k == NKT - 1),
                                )
                                nc.tensor.matmul(
                                    pn[:], esl, ones[:],
                                    start=(k == 0), stop=(k == NKT - 1),
                                )
                            rs = attn.tile([P, 1], F32, tag="rs", bufs=2,
                                           name="rs")
                            nc.vector.reciprocal(rs[:], pn[:, 0:1])
                            ot = attn.tile([P, HID], F32, tag="ot", bufs=2,
                                           name="ot")
                            nc.vector.tensor_scalar_mul(ot[:, 0:QC], po0[:],
                                                        rs[:])
                            nc.vector.tensor_scalar_mul(ot[:, QC:HID], po1[:],
                                                        rs[:])
                            nc.vector.tensor_add(ot[:], ot[:], bv_bc[:])
                            row = qc * QC + qs * P
                            nc.sync.dma_start(out=out[row:row + P, :],
                                              in_=ot[:])

    nc.compile()
    return nc


def _get_nc():
    if "nc" not in _CACHED:
        _CACHED["nc"] = build_kernel()
    return _CACHED["nc"]


def kernel(image_features, text_features, Wq, bq, Wk, bk, Wv, bv):
    bf = ml_dtypes.bfloat16
    img = np.ascontiguousarray(np.asarray(image_features).astype(bf))
    txt = np.ascontiguousarray(np.asarray(text_features).astype(bf))
    shared = {
        "wq": np.ascontiguousarray(np.asarray(Wq).astype(bf)),
        "wk": np.ascontiguousarray(np.asarray(Wk).astype(bf)),
        "wv": np.ascontiguousarray(np.asarray(Wv).astype(bf)),
        "bq": np.ascontiguousarray(np.asarray(bq, np.float32)),
        "bv": np.ascontiguousarray(np.asarray(bv, np.float32)),
    }
    in_maps = [{"img": img[b], "txt": txt[b], **shared} for b in range(B)]
    res = run_bass_kernel_spmd(_get_nc(), in_maps, core_ids=list(range(B)))
    return np.stack([res.results[b]["out_attn"] for b in range(B)])
